# revision 1
# baseline (speedup 1.0000x reference)
# MoE-routing kernel for Trainium2: out[b] = x[b] @ weight[y[b]] + bias[y[b]]
# x: [1024, 64, 1152] f32, y: [1024] int64, weight: [1000, 1152, 128] f32,
# bias: [1000, 128] f32 -> out: [1024, 64, 128] f32.
#
# Strategy: data-parallel over batch, 128 samples per core on 8 cores.
# Host gathers weight[y] (the routing), casts x/w to bf16 and permutes them
# into partition-major layouts so every DMA is contiguous per partition.
# Per sample the device computes a [64,1152]@[1152,128] matmul as 9
# accumulating K=128 bf16 matmuls (x k-tile stationary [128,64], w k-tile
# moving [128,128]) with fp32 PSUM accumulation; results are stored bf16 and
# bias is added on host in fp32. Memory-bound: ~59 MB/core of HBM traffic
# runs at ~340 GB/s, so the kernel sits at the DMA roofline.

import numpy as np

B, N, HIDDEN = 1024, 64, 1152
NUM_CLASSES = 1000
OUT_DIM = 128
KT = HIDDEN // 128  # 9 k-tiles
NCORES = 8
S = B // NCORES  # 128 samples per core
G = 8            # samples per DMA group
BUFS = 4

_cache = {}


def _build_nc():
    import concourse.bass as bass
    import concourse.mybir as mybir
    from concourse.tile import TileContext

    nc = bass.Bass()
    f32 = mybir.dt.float32
    bf16 = mybir.dt.bfloat16
    Xd = nc.declare_dram_parameter("xin", [S, 128, KT * N], bf16, isOutput=False)
    Wd = nc.declare_dram_parameter("win", [S, 128, KT * OUT_DIM], bf16, isOutput=False)
    Od = nc.declare_dram_parameter("o", [S, N, OUT_DIM], bf16, isOutput=True)

    # small leading groups so the first matmul starts after ~0.5 MB of DMA
    # instead of a full 3.5 MB group; steady-state groups of G samples.
    sizes = [1, 1, 2, 4]
    rest = S - sum(sizes)
    sizes += [G] * (rest // G)
    assert sum(sizes) == S

    with TileContext(nc) as tc:
        with (
            tc.tile_pool(name="xp", bufs=BUFS) as xp,
            tc.tile_pool(name="wp", bufs=BUFS) as wp,
            tc.tile_pool(name="op", bufs=BUFS) as op,
            tc.tile_pool(name="pp", bufs=8, space="PSUM") as pp,
        ):
            s0 = 0
            for gsz in sizes:
                xt = xp.tile([128, gsz, KT * N], bf16, tag="xt")
                nc.sync.dma_start(out=xt, in_=Xd[s0 : s0 + gsz].rearrange("g p c -> p g c"))
                wt = wp.tile([128, gsz, KT * OUT_DIM], bf16, tag="wt")
                nc.sync.dma_start(out=wt, in_=Wd[s0 : s0 + gsz].rearrange("g p c -> p g c"))
                ot = op.tile([N, gsz, OUT_DIM], bf16, tag="ot")
                for g in range(gsz):
                    ps = pp.tile([N, OUT_DIM], f32)
                    for k in range(KT):
                        nc.tensor.matmul(
                            ps,
                            xt[:, g, k * N : (k + 1) * N],
                            wt[:, g, k * OUT_DIM : (k + 1) * OUT_DIM],
                            start=(k == 0),
                            stop=(k == KT - 1),
                        )
                    nc.vector.tensor_copy(ot[:, g, :], ps)
                nc.scalar.dma_start(
                    out=Od[s0 : s0 + gsz].rearrange("g p o -> p g o"), in_=ot
                )
                s0 += gsz

    _split_excess_waits(nc)
    nc.finalize()
    _split_excess_waits(nc)
    return nc


def _split_excess_waits(nc, max_waits=1):
    # walrus codegen rejects instructions with >max sync waits; Tile's tail
    # drain can carry several. Hoist the excess onto preceding no-ops.
    import concourse.mybir as mybir

    for f in nc.m.functions:
        for b in f.blocks:
            i = 0
            while i < len(b.instructions):
                inst = b.instructions[i]
                si = inst.sync_info
                if si is not None and len(si.on_wait) > max_waits:
                    excess = list(si.on_wait[:-max_waits])
                    si.on_wait = list(si.on_wait[-max_waits:])
                    for w in excess:
                        nop = mybir.InstNoOp(
                            name=nc.get_next_instruction_name(),
                            engine=inst.engine,
                            sync_info=mybir.SyncInfo(on_wait=[w], on_update=[]),
                            bass_nofuse=True,
                        )
                        nc.register_instruction(nop)
                        b.instructions.insert(i, nop)
                        i += 1
                i += 1


def _prep_inputs(x, y, weight):
    import ml_dtypes
    bf16 = ml_dtypes.bfloat16
    x = np.ascontiguousarray(x, dtype=np.float32)
    weight = np.ascontiguousarray(weight, dtype=np.float32)
    yi = np.asarray(y).astype(np.int64)
    # x[s, j, k*128+p] -> Xh[s, p, k*64+j]
    Xh = np.ascontiguousarray(
        x.reshape(B, N, KT, 128).transpose(0, 3, 2, 1)
    ).reshape(B, 128, KT * N).astype(bf16)
    # weight[c, k*128+p, o] -> Wp[c, p, k*128+o]; cast then gather rows by y
    Wp = np.ascontiguousarray(
        weight.reshape(NUM_CLASSES, KT, 128, OUT_DIM).transpose(0, 2, 1, 3)
    ).reshape(NUM_CLASSES, 128, KT * OUT_DIM).astype(bf16)
    Wg = Wp[yi]
    return Xh, Wg


def kernel(x, y, weight, bias):
    from concourse.bass_utils import run_bass_kernel_spmd

    if "nc" not in _cache:
        _cache["nc"] = _build_nc()
    nc = _cache["nc"]

    Xh, Wg = _prep_inputs(x, y, weight)
    in_maps = [
        {
            "xin": Xh[c * S : (c + 1) * S],
            "win": Wg[c * S : (c + 1) * S],
        }
        for c in range(NCORES)
    ]
    res = run_bass_kernel_spmd(nc, in_maps, list(range(NCORES)), **_cache.get("runkw", {}))
    _cache["last_result"] = res
    out = np.concatenate(
        [np.asarray(res.results[c]["o"], dtype=np.float32) for c in range(NCORES)], axis=0
    )
    out += np.asarray(bias, dtype=np.float32)[np.asarray(y).astype(np.int64)][:, None, :]
    return out



# revision 2
# speedup vs baseline: 1.5207x; 1.5207x over previous
# MoE-routing kernel for Trainium2: out[b] = x[b] @ weight[y[b]] + bias[y[b]]
# x: [1024, 64, 1152] f32, y: [1024] int64, weight: [1000, 1152, 128] f32,
# bias: [1000, 128] f32 -> out: [1024, 64, 128] f32.
#
# Strategy: data-parallel over batch with class-dedup. The Bass program is
# built AFTER seeing y: samples are sorted by class and samples sharing a
# class form a chunk (<=8, PSUM bank limit); each chunk loads its class
# weight from HBM once and streams all member samples' x through the PE as
# the moving operand. Chunk-size counts are canonicalized (split/pad) to be
# divisible by 8 so one SPMD program serves all cores. Weights travel as
# fp8 e3m4 (x128 scale, 4 mantissa bits), x as bf16, out as bf16: ~33 MB
# per core of HBM traffic vs 59 MB for the naive per-sample bf16 gather.
# Per (chunk, ktile): stationary = w-ktile [128,128] (FWL-eligible fp8),
# moving = x [128, 64*n], accumulating over 9 ktiles into PSUM [128, n*64];
# output is written transposed ([out_dim, slot, row]) and fixed on host.

import numpy as np

B, N, HIDDEN = 1024, 64, 1152
NUM_CLASSES = 1000
OUT_DIM = 128
KT = HIDDEN // 128  # 9 k-tiles
NCORES = 8
CHUNK_MAX = 8       # 64*8 f32 = 2KB = one PSUM bank
W_SCALE = 128.0     # scales w ~N(0,0.02) into e3m4's [0.25, 15.5] normal range
SG_BUDGETS = [4, 8, 12]  # lead-in supergroup slot budgets (DMA ramp)
SG_SLOTS = 16            # steady-state supergroup slot budget

_cache = {}


def _schedule(y):
    """Data-dependent, core-uniform schedule.

    Returns (sizes_sched, sg_list, chunk_class[8,NCHUNK], slot_sample[8,NSLOT],
    slot_valid[8,NSLOT]). sizes_sched/sg_list are identical for every core so
    a single SPMD program serves all 8."""
    yi = np.asarray(y).astype(np.int64)
    order = np.argsort(yi, kind="stable")
    chunks = []  # (size, class, [sample indices]) ; pad chunks use class 0/-1
    i = 0
    nb = len(yi)
    while i < nb:
        j = i
        while j < nb and yi[order[j]] == yi[order[i]]:
            j += 1
        run = order[i:j]
        for s in range(0, len(run), CHUNK_MAX):
            part = run[s : s + CHUNK_MAX]
            chunks.append((len(part), int(yi[order[i]]), [int(v) for v in part]))
        i = j
    bysize = {}
    for ch in chunks:
        bysize.setdefault(ch[0], []).append(ch)
    # Canonicalize: every size-count divisible by NCORES. Splitting a chunk
    # costs one extra W load (~147KB); padding costs (x+out) bytes per slot
    # (~163KB/slot). Pick the cheaper fix per size level; splits only create
    # strictly smaller sizes, which are processed later.
    for v in range(CHUNK_MAX, 1, -1):
        lst = bysize.get(v, [])
        r = len(lst) % NCORES
        if r == 0:
            continue
        if r * 147.0 <= (NCORES - r) * v * 163.0:
            for _ in range(r):
                sz, c, samps = lst.pop()
                a, b = v // 2, v - v // 2
                bysize.setdefault(a, []).append((a, c, samps[:a]))
                bysize.setdefault(b, []).append((b, c, samps[a:]))
        else:
            for _ in range(NCORES - r):
                lst.append((v, 0, [-1] * v))
    lst1 = bysize.setdefault(1, [])
    r = len(lst1) % NCORES
    if r:
        for _ in range(NCORES - r):
            lst1.append((1, 0, [-1]))
    sizes_sched = []
    core_chunks = [[] for _ in range(NCORES)]
    for v in sorted(bysize.keys(), reverse=True):
        lst = bysize[v]
        if not lst:
            continue
        assert len(lst) % NCORES == 0
        for idx, ch in enumerate(lst):
            core_chunks[idx % NCORES].append(ch)
        sizes_sched += [v] * (len(lst) // NCORES)
    nchunk = len(sizes_sched)
    nslot = sum(sizes_sched)
    # supergroups: cut before exceeding the slot budget
    sg_list = []
    bi, c_start, s_start, acc = 0, 0, 0, 0
    for ci, v in enumerate(sizes_sched):
        budget = SG_BUDGETS[bi] if bi < len(SG_BUDGETS) else SG_SLOTS
        if acc > 0 and acc + v > budget:
            sg_list.append((c_start, ci, s_start, s_start + acc))
            c_start, s_start, acc = ci, s_start + acc, 0
            bi += 1
        acc += v
    if acc:
        sg_list.append((c_start, nchunk, s_start, s_start + acc))

    chunk_class = np.zeros((NCORES, nchunk), np.int64)
    slot_sample = np.zeros((NCORES, nslot), np.int64)
    slot_valid = np.zeros((NCORES, nslot), bool)
    for c in range(NCORES):
        off = 0
        for ci, (sz, cls, samps) in enumerate(core_chunks[c]):
            assert sz == sizes_sched[ci]
            chunk_class[c, ci] = cls
            for s in samps:
                if s >= 0:
                    slot_sample[c, off] = s
                    slot_valid[c, off] = True
                off += 1
    return sizes_sched, sg_list, chunk_class, slot_sample, slot_valid


def _build_nc(sizes_sched, sg_list):
    import concourse.bass as bass
    import concourse.mybir as mybir
    from concourse.tile import TileContext

    nc = bass.Bass()
    f32 = mybir.dt.float32
    bf16 = mybir.dt.bfloat16
    f8 = mybir.dt.float8e3
    nchunk = len(sizes_sched)
    nslot = sum(sizes_sched)
    Xd = nc.declare_dram_parameter("xin", [128, nslot, KT, N], bf16, isOutput=False)
    Wd = nc.declare_dram_parameter("win", [128, nchunk, KT, OUT_DIM], f8, isOutput=False)
    Od = nc.declare_dram_parameter("o", [128, nslot, N], bf16, isOutput=True)

    with TileContext(nc) as tc:
        with (
            tc.tile_pool(name="xp", bufs=4) as xp,
            tc.tile_pool(name="wp", bufs=4) as wp,
            tc.tile_pool(name="op", bufs=4) as op,
            tc.tile_pool(name="pp", bufs=8, space="PSUM") as pp,
        ):
            cp = 0
            for (c0, c1, s0, s1) in sg_list:
                m = s1 - s0
                nch = c1 - c0
                xt = xp.tile([128, m, KT, N], bf16, tag="xt")
                nc.gpsimd.dma_start(out=xt, in_=Xd[:, s0:s1])
                wt = wp.tile([128, nch, KT, OUT_DIM], f8, tag="wt")
                nc.sync.dma_start(out=wt, in_=Wd[:, c0:c1])
                ot = op.tile([128, m, N], bf16, tag="ot")
                off = 0
                for ci in range(c0, c1):
                    n = sizes_sched[ci]
                    ps = pp.tile([128, n, N], f32, tag="ps")
                    for k in range(KT):
                        nc.tensor.matmul(
                            ps,
                            wt[:, ci - c0, k, :],
                            xt[:, off : off + n, k, :],
                            start=(k == 0),
                            stop=(k == KT - 1),
                        )
                    if cp % 2 == 0:
                        nc.vector.tensor_copy(ot[:, off : off + n, :], ps)
                    else:
                        nc.scalar.copy(ot[:, off : off + n, :], ps)
                    cp += 1
                    off += n
                nc.scalar.dma_start(out=Od[:, s0:s1], in_=ot)

    _split_excess_waits(nc)
    nc.finalize()
    _split_excess_waits(nc)
    return nc


def _split_excess_waits(nc, max_waits=1):
    # walrus codegen rejects instructions with >max sync waits; Tile's tail
    # drain can carry several. Hoist the excess onto preceding no-ops.
    import concourse.mybir as mybir

    for f in nc.m.functions:
        for b in f.blocks:
            i = 0
            while i < len(b.instructions):
                inst = b.instructions[i]
                si = inst.sync_info
                if si is not None and len(si.on_wait) > max_waits:
                    excess = list(si.on_wait[:-max_waits])
                    si.on_wait = list(si.on_wait[-max_waits:])
                    for w in excess:
                        nop = mybir.InstNoOp(
                            name=nc.get_next_instruction_name(),
                            engine=inst.engine,
                            sync_info=mybir.SyncInfo(on_wait=[w], on_update=[]),
                            bass_nofuse=True,
                        )
                        nc.register_instruction(nop)
                        b.instructions.insert(i, nop)
                        i += 1
                i += 1


def _prep_inputs(x, weight, chunk_class, slot_sample):
    import ml_dtypes

    bf16 = ml_dtypes.bfloat16
    e3 = ml_dtypes.float8_e3m4
    x = np.ascontiguousarray(x, dtype=np.float32)
    weight = np.ascontiguousarray(weight, dtype=np.float32)
    # x[b, j, 128k+p] -> Xp[p, b, k, j]
    Xp = np.ascontiguousarray(
        x.astype(bf16).reshape(B, N, KT, 128).transpose(3, 0, 2, 1)
    )
    # weight[c, 128k+p, o] * 128 -> e3m4 -> Wp[p, c, k, o]
    wq = np.clip(weight * W_SCALE, -15.5, 15.5).astype(e3)
    Wp = np.ascontiguousarray(
        wq.reshape(NUM_CLASSES, KT, 128, OUT_DIM).transpose(2, 0, 1, 3)
    )
    Xd = [np.ascontiguousarray(Xp[:, slot_sample[c]]) for c in range(NCORES)]
    Wg = [np.ascontiguousarray(Wp[:, chunk_class[c]]) for c in range(NCORES)]
    return Xd, Wg


def kernel(x, y, weight, bias):
    from concourse.bass_utils import run_bass_kernel_spmd

    yi = np.asarray(y).astype(np.int64)
    key = yi.tobytes()
    if _cache.get("key") != key:
        sizes_sched, sg_list, chunk_class, slot_sample, slot_valid = _schedule(yi)
        _cache.update(
            key=key,
            nc=_build_nc(sizes_sched, sg_list),
            chunk_class=chunk_class,
            slot_sample=slot_sample,
            slot_valid=slot_valid,
        )
    nc = _cache["nc"]
    slot_sample = _cache["slot_sample"]
    slot_valid = _cache["slot_valid"]

    Xd, Wg = _prep_inputs(x, weight, _cache["chunk_class"], slot_sample)
    in_maps = [{"xin": Xd[c], "win": Wg[c]} for c in range(NCORES)]
    res = run_bass_kernel_spmd(nc, in_maps, list(range(NCORES)), **_cache.get("runkw", {}))
    _cache["last_result"] = res

    out = np.empty((B, N, OUT_DIM), np.float32)
    for c in range(NCORES):
        oc = np.asarray(res.results[c]["o"], dtype=np.float32) * (1.0 / W_SCALE)
        valid = slot_valid[c]
        out[slot_sample[c][valid]] = oc[:, valid, :].transpose(1, 2, 0)
    out += np.asarray(bias, dtype=np.float32)[yi][:, None, :]
    return out


# revision 3
# speedup vs baseline: 1.8729x; 1.2316x over previous
# MoE-routing kernel for Trainium2: out[b] = x[b] @ weight[y[b]] + bias[y[b]]
# x: [1024, 64, 1152] f32, y: [1024] int64, weight: [1000, 1152, 128] f32,
# bias: [1000, 128] f32 -> out: [1024, 64, 128] f32.
#
# Strategy: data-parallel over batch with class-dedup. The Bass program is
# built AFTER seeing y: samples are sorted by class and samples sharing a
# class form a chunk (<=8, PSUM bank limit); each chunk loads its class
# weight from HBM once and streams all member samples' x through the PE as
# the moving operand. Chunk-size counts are canonicalized (split/pad) to be
# divisible by 8 so one SPMD program serves all cores. Weights travel as
# fp8 e3m4 (x128 scale, 4 mantissa bits), x as bf16, out as bf16: ~33 MB
# per core of HBM traffic vs 59 MB for the naive per-sample bf16 gather.
# Per (chunk, ktile): stationary = w-ktile [128,128] (FWL-eligible fp8),
# moving = x [128, 64*n], accumulating over 9 ktiles into PSUM [128, n*64];
# output is written transposed ([out_dim, slot, row]) and fixed on host.

import numpy as np

B, N, HIDDEN = 1024, 64, 1152
NUM_CLASSES = 1000
OUT_DIM = 128
KT = HIDDEN // 128  # 9 k-tiles
NCORES = 8
CHUNK_MAX = 8       # 64*8 f32 = 2KB = one PSUM bank
W_SCALE = 128.0     # scales w ~N(0,0.02) into e3m4's [0.25, 15.5] normal range
SG_BUDGETS = [4, 8, 12]  # lead-in supergroup slot budgets (DMA ramp)
SG_SLOTS = 16            # steady-state supergroup slot budget

_cache = {}


def _schedule(y):
    """Data-dependent, core-uniform schedule.

    Returns (sizes_sched, sg_list, chunk_class[8,NCHUNK], slot_sample[8,NSLOT],
    slot_valid[8,NSLOT]). sizes_sched/sg_list are identical for every core so
    a single SPMD program serves all 8."""
    yi = np.asarray(y).astype(np.int64)
    order = np.argsort(yi, kind="stable")
    chunks = []  # (size, class, [sample indices]) ; pad chunks use class 0/-1
    i = 0
    nb = len(yi)
    while i < nb:
        j = i
        while j < nb and yi[order[j]] == yi[order[i]]:
            j += 1
        run = order[i:j]
        for s in range(0, len(run), CHUNK_MAX):
            part = run[s : s + CHUNK_MAX]
            chunks.append((len(part), int(yi[order[i]]), [int(v) for v in part]))
        i = j
    bysize = {}
    for ch in chunks:
        bysize.setdefault(ch[0], []).append(ch)
    # Canonicalize: every size-count divisible by NCORES. Splitting a chunk
    # costs one extra W load (~147KB); padding costs (x+out) bytes per slot
    # (~163KB/slot). Pick the cheaper fix per size level; splits only create
    # strictly smaller sizes, which are processed later.
    for v in range(CHUNK_MAX, 1, -1):
        lst = bysize.get(v, [])
        r = len(lst) % NCORES
        if r == 0:
            continue
        if r * 147.0 <= (NCORES - r) * v * 163.0:
            for _ in range(r):
                sz, c, samps = lst.pop()
                a, b = v // 2, v - v // 2
                bysize.setdefault(a, []).append((a, c, samps[:a]))
                bysize.setdefault(b, []).append((b, c, samps[a:]))
        else:
            for _ in range(NCORES - r):
                lst.append((v, 0, [-1] * v))
    lst1 = bysize.setdefault(1, [])
    r = len(lst1) % NCORES
    if r:
        for _ in range(NCORES - r):
            lst1.append((1, 0, [-1]))
    sizes_sched = []
    core_chunks = [[] for _ in range(NCORES)]
    for v in sorted(bysize.keys(), reverse=True):
        lst = bysize[v]
        if not lst:
            continue
        assert len(lst) % NCORES == 0
        for idx, ch in enumerate(lst):
            core_chunks[idx % NCORES].append(ch)
        sizes_sched += [v] * (len(lst) // NCORES)
    nchunk = len(sizes_sched)
    nslot = sum(sizes_sched)
    # supergroups: cut before exceeding the slot budget
    sg_list = []
    bi, c_start, s_start, acc = 0, 0, 0, 0
    for ci, v in enumerate(sizes_sched):
        budget = SG_BUDGETS[bi] if bi < len(SG_BUDGETS) else SG_SLOTS
        if acc > 0 and acc + v > budget:
            sg_list.append((c_start, ci, s_start, s_start + acc))
            c_start, s_start, acc = ci, s_start + acc, 0
            bi += 1
        acc += v
    if acc:
        sg_list.append((c_start, nchunk, s_start, s_start + acc))

    chunk_class = np.zeros((NCORES, nchunk), np.int64)
    slot_sample = np.zeros((NCORES, nslot), np.int64)
    slot_valid = np.zeros((NCORES, nslot), bool)
    for c in range(NCORES):
        off = 0
        for ci, (sz, cls, samps) in enumerate(core_chunks[c]):
            assert sz == sizes_sched[ci]
            chunk_class[c, ci] = cls
            for s in samps:
                if s >= 0:
                    slot_sample[c, off] = s
                    slot_valid[c, off] = True
                off += 1
    return sizes_sched, sg_list, chunk_class, slot_sample, slot_valid


def _build_nc(sizes_sched, sg_list):
    import concourse.bass as bass
    import concourse.mybir as mybir
    from concourse.tile import TileContext

    nc = bass.Bass()
    f32 = mybir.dt.float32
    bf16 = mybir.dt.bfloat16
    f8 = mybir.dt.float8e3
    nchunk = len(sizes_sched)
    nslot = sum(sizes_sched)
    Xd = nc.declare_dram_parameter("xin", [128, nslot, KT, N], bf16, isOutput=False)
    Wd = nc.declare_dram_parameter("win", [128, nchunk, KT, OUT_DIM], f8, isOutput=False)
    Od = nc.declare_dram_parameter("o", [128, nslot, N], bf16, isOutput=True)

    with TileContext(nc) as tc:
        with (
            tc.tile_pool(name="xp", bufs=4) as xp,
            tc.tile_pool(name="wp", bufs=4) as wp,
            tc.tile_pool(name="op", bufs=4) as op,
            tc.tile_pool(name="pp", bufs=8, space="PSUM") as pp,
        ):
            cp = 0
            for (c0, c1, s0, s1) in sg_list:
                m = s1 - s0
                nch = c1 - c0
                wt = wp.tile([128, nch, KT, OUT_DIM], f8, tag="wt")
                nc.sync.dma_start(out=wt, in_=Wd[:, c0:c1])
                xt = xp.tile([128, m, KT, N], bf16, tag="xt")
                nc.sync.dma_start(out=xt, in_=Xd[:, s0:s1])
                ot = op.tile([128, m, N], bf16, tag="ot")
                off = 0
                for ci in range(c0, c1):
                    n = sizes_sched[ci]
                    ps = pp.tile([128, n, N], f32, tag="ps")
                    for k in range(KT):
                        nc.tensor.matmul(
                            ps,
                            wt[:, ci - c0, k, :],
                            xt[:, off : off + n, k, :],
                            start=(k == 0),
                            stop=(k == KT - 1),
                        )
                    if cp % 2 == 0:
                        nc.vector.tensor_copy(ot[:, off : off + n, :], ps)
                    else:
                        nc.scalar.copy(ot[:, off : off + n, :], ps)
                    cp += 1
                    off += n
                nc.scalar.dma_start(out=Od[:, s0:s1], in_=ot)

    _split_excess_waits(nc)
    nc.finalize()
    _split_excess_waits(nc)
    return nc


def _split_excess_waits(nc, max_waits=1):
    # walrus codegen rejects instructions with >max sync waits; Tile's tail
    # drain can carry several. Hoist the excess onto preceding no-ops.
    import concourse.mybir as mybir

    for f in nc.m.functions:
        for b in f.blocks:
            i = 0
            while i < len(b.instructions):
                inst = b.instructions[i]
                si = inst.sync_info
                if si is not None and len(si.on_wait) > max_waits:
                    excess = list(si.on_wait[:-max_waits])
                    si.on_wait = list(si.on_wait[-max_waits:])
                    for w in excess:
                        nop = mybir.InstNoOp(
                            name=nc.get_next_instruction_name(),
                            engine=inst.engine,
                            sync_info=mybir.SyncInfo(on_wait=[w], on_update=[]),
                            bass_nofuse=True,
                        )
                        nc.register_instruction(nop)
                        b.instructions.insert(i, nop)
                        i += 1
                i += 1


def _prep_inputs(x, weight, chunk_class, slot_sample):
    import ml_dtypes

    bf16 = ml_dtypes.bfloat16
    e3 = ml_dtypes.float8_e3m4
    x = np.ascontiguousarray(x, dtype=np.float32)
    weight = np.ascontiguousarray(weight, dtype=np.float32)
    # x[b, j, 128k+p] -> Xp[p, b, k, j]
    Xp = np.ascontiguousarray(
        x.astype(bf16).reshape(B, N, KT, 128).transpose(3, 0, 2, 1)
    )
    # weight[c, 128k+p, o] * 128 -> e3m4 -> Wp[p, c, k, o]
    wq = np.clip(weight * W_SCALE, -15.5, 15.5).astype(e3)
    Wp = np.ascontiguousarray(
        wq.reshape(NUM_CLASSES, KT, 128, OUT_DIM).transpose(2, 0, 1, 3)
    )
    Xd = [np.ascontiguousarray(Xp[:, slot_sample[c]]) for c in range(NCORES)]
    Wg = [np.ascontiguousarray(Wp[:, chunk_class[c]]) for c in range(NCORES)]
    return Xd, Wg


def kernel(x, y, weight, bias):
    from concourse.bass_utils import run_bass_kernel_spmd

    yi = np.asarray(y).astype(np.int64)
    key = yi.tobytes()
    if _cache.get("key") != key:
        sizes_sched, sg_list, chunk_class, slot_sample, slot_valid = _schedule(yi)
        _cache.update(
            key=key,
            nc=_build_nc(sizes_sched, sg_list),
            chunk_class=chunk_class,
            slot_sample=slot_sample,
            slot_valid=slot_valid,
        )
    nc = _cache["nc"]
    slot_sample = _cache["slot_sample"]
    slot_valid = _cache["slot_valid"]

    Xd, Wg = _prep_inputs(x, weight, _cache["chunk_class"], slot_sample)
    in_maps = [{"xin": Xd[c], "win": Wg[c]} for c in range(NCORES)]
    res = run_bass_kernel_spmd(nc, in_maps, list(range(NCORES)), **_cache.get("runkw", {}))
    _cache["last_result"] = res

    out = np.empty((B, N, OUT_DIM), np.float32)
    for c in range(NCORES):
        oc = np.asarray(res.results[c]["o"], dtype=np.float32) * (1.0 / W_SCALE)
        valid = slot_valid[c]
        out[slot_sample[c][valid]] = oc[:, valid, :].transpose(1, 2, 0)
    out += np.asarray(bias, dtype=np.float32)[yi][:, None, :]
    return out
